# revision 13
# baseline (speedup 1.0000x reference)
"""Trainium2 Bass kernel for nn_Attentioncross (gnn_message_passing).

Reference computation, per node n (N=50000) and row r (R=8), D=256:
    idx[r] = [r, r+1, r-1] (with idx[0]=[0,1,2], idx[7]=[7,6,5])
    s[n,j]   = W2 @ leaky_relu(W1 @ z[n,j,:], 0.01)        (scalar per row)
    beta     = softmax([s[self], s[j1], s[j2]])            (over the 3)
    o[n,r,:] = z[n,r,:] + beta1*z[n,j1,:] + beta2*z[n,j2,:]

Strategy: data-parallel over N across 8 cores. Per core, rows are tiled
[128, 256] (16 nodes/tile; the neighbor structure is block-diagonal per
node, so intra-tile). The score path consumes a bf16 copy of z loaded
pre-transposed via the DMA xbar (d on partitions), so no PE transposes
are needed: h^T = W1 @ z^T as bf16 matmuls, leaky on ScalarE, then
s = h^T.T @ W2^T gives per-row scores on partitions. Neighbor gathers of
scores and of z-rows are f32r matmuls against static block-diagonal 0/1
shift matrices; softmax runs on [128, 8] vectors; the combine applies
per-row betas via scalar_tensor_tensor / scaled-copy with the fp32
residual added exactly.
"""
import sys

for p in ("/opt/trn_rl_repo",):
    if p not in sys.path:
        sys.path.insert(0, p)

import numpy as np
from contextlib import ExitStack

N_FULL, R, D = 50000, 8, 256
N_CORES = 8
NODES_PER_TILE = 16          # 128 rows / 8
P = 128
TILES_PER_CORE = 391         # 6256 nodes/core * 8 rows / 128
NODES_PER_CORE = TILES_PER_CORE * NODES_PER_TILE   # 6256
N_PAD = NODES_PER_CORE * N_CORES                   # 50048
ROWS_PER_CORE = NODES_PER_CORE * R                 # 50048
SUPER = 16                   # tiles per supertile

# static 3-neighbor pattern (matches reference._neighbor_idx for R=8)
J1 = [1, 2, 3, 4, 5, 6, 7, 6]
J2 = [2, 0, 1, 2, 3, 4, 5, 5]


def _build_consts(W1, W2):
    g1 = np.zeros((P, P), np.float16)
    g2 = np.zeros((P, P), np.float16)
    for b in range(NODES_PER_TILE):
        for r in range(R):
            g1[b * R + J1[r], b * R + r] = 1.0
            g2[b * R + J2[r], b * R + r] = 1.0
    w1t = np.ascontiguousarray(
        W1.T.reshape(2, P, 16).transpose(1, 0, 2)
    ).astype(np.float16)  # [128, 2, 16]
    w2t = np.ascontiguousarray(
        np.repeat(W2.reshape(16, 1), 2, axis=1)
    ).astype(np.float16)  # [16, 2]
    return g1, g2, w1t, w2t


def _build_nc():
    import concourse.bacc as bacc
    import concourse.tile as tile
    from concourse import mybir

    f32 = mybir.dt.float32
    f32r = mybir.dt.float32r
    f16 = mybir.dt.float16

    nc = bacc.Bacc("TRN2", target_bir_lowering=False)
    z_d = nc.declare_dram_parameter("z", [ROWS_PER_CORE, D], f16, isOutput=False)
    zt_d = nc.declare_dram_parameter("zt", [2, P, ROWS_PER_CORE], f16, isOutput=False)
    g1_d = nc.declare_dram_parameter("g1", [P, P], f16, isOutput=False)
    g2_d = nc.declare_dram_parameter("g2", [P, P], f16, isOutput=False)
    w1t_d = nc.declare_dram_parameter("w1t", [P, 2, 16], f16, isOutput=False)
    w2t_d = nc.declare_dram_parameter("w2t", [16, 2], f16, isOutput=False)
    o_d = nc.declare_dram_parameter("o", [ROWS_PER_CORE, D], f32, isOutput=True)

    Prelu = mybir.ActivationFunctionType.Prelu
    Exp = mybir.ActivationFunctionType.Exp
    Copy = mybir.ActivationFunctionType.Copy
    add = mybir.AluOpType.add
    mult = mybir.AluOpType.mult

    with tile.TileContext(nc) as tc, ExitStack() as ctx:
        consts = ctx.enter_context(tc.tile_pool(name="consts", bufs=1))
        zpool = ctx.enter_context(tc.tile_pool(name="zp", bufs=4))
        opool = ctx.enter_context(tc.tile_pool(name="op", bufs=3))
        zt_pool = ctx.enter_context(tc.tile_pool(name="ztp", bufs=3))
        small = ctx.enter_context(tc.tile_pool(name="small", bufs=3))
        tzpool = ctx.enter_context(tc.tile_pool(name="tzp", bufs=6))

        ps_ht = ctx.enter_context(tc.tile_pool(name="ps_ht", bufs=1, space="PSUM"))
        ps_sc = ctx.enter_context(tc.tile_pool(name="ps_sc", bufs=1, space="PSUM"))
        ps_zsh1 = ctx.enter_context(tc.tile_pool(name="ps_zsh1", bufs=3, space="PSUM"))
        ps_zsh2 = ctx.enter_context(tc.tile_pool(name="ps_zsh2", bufs=3, space="PSUM"))

        g1_sb = consts.tile([P, P], f16)
        g2_sb = consts.tile([P, P], f16)
        w1t_sb = consts.tile([P, 2, 16], f16)
        w2t_sb = consts.tile([16, 2], f16)
        nc.sync.dma_start(out=g1_sb, in_=g1_d[:])
        nc.sync.dma_start(out=g2_sb, in_=g2_d[:])
        nc.sync.dma_start(out=w1t_sb, in_=w1t_d[:])
        nc.sync.dma_start(out=w2t_sb, in_=w2t_d[:])

        n_super = (TILES_PER_CORE + SUPER - 1) // SUPER
        for st in range(n_super):
            g0 = st * SUPER
            G = min(SUPER, TILES_PER_CORE - g0)
            row0 = g0 * P

            z_sb = zpool.tile([P, G, D], f16, tag="z")
            nc.sync.dma_start(
                out=z_sb,
                in_=z_d[row0 : row0 + G * P, :].rearrange(
                    "(g p) d -> p g d", p=P
                ),
            )
            # fp16 z^T for the score path (host-pretransposed, contiguous)
            zt_sb = zt_pool.tile([P, 2, SUPER * P], f16, tag="zt")
            nc.scalar.dma_start(
                out=zt_sb[:, :, 0 : G * P],
                in_=zt_d[:, :, row0 : row0 + G * P].rearrange("c p r -> p c r"),
            )

            # ---- phase A: scores ----
            nquad = (G + 3) // 4
            ht_sbs = []
            for q in range(nquad):
                m = min(4, G - q * 4)  # tiles in this quad
                ht_ps = ps_ht.tile([16, 4, P], f32, tag="ht")
                for c in range(2):
                    nc.tensor.matmul(
                        ht_ps[:, 0:m, :],
                        w1t_sb[:, c, :],
                        zt_sb[:, c, q * 4 * P : (q * 4 + m) * P],
                        start=(c == 0),
                        stop=(c == 1),
                    )
                ht_sb = small.tile([16, 4, P], f16, tag=f"ht{q % 2}")
                nc.scalar.activation(
                    ht_sb[:, 0:m, :], ht_ps[:, 0:m, :], Prelu, alpha=0.01
                )
                ht_sbs.append((ht_sb, m))

            # scores psum bank: cols 0:16 = s ([8 tiles, 2]), 16:32 = s1/s2
            sc_ps = ps_sc.tile([P, 2 * SUPER + 2 * SUPER], f32, tag="sc")
            for q, (ht_sb, m) in enumerate(ht_sbs):
                for t in range(m):
                    g = q * 4 + t
                    nc.tensor.matmul(
                        sc_ps[:, 2 * g : 2 * g + 2],
                        ht_sb[:, t, :],
                        w2t_sb,
                        start=True,
                        stop=True,
                    )
            s_sb = small.tile([P, SUPER], f16, tag="ssb")
            nc.vector.tensor_copy(
                s_sb,
                sc_ps[:, 0 : 2 * SUPER].rearrange("p (g two) -> p g two", two=2)[
                    :, :, 0
                ],
            )
            nc.tensor.matmul(sc_ps[:, 2 * SUPER : 3 * SUPER], g1_sb, s_sb, start=True, stop=True)
            nc.tensor.matmul(sc_ps[:, 3 * SUPER : 4 * SUPER], g2_sb, s_sb, start=True, stop=True)

            # softmax over {self, n1, n2}; no max-subtraction (|s| < ~8)
            e0 = small.tile([P, SUPER], f32, tag="e0")
            e12 = small.tile([P, 2, SUPER], f32, tag="e12")
            nc.scalar.activation(e0, s_sb, Exp)
            nc.scalar.activation(
                e12, sc_ps[:, 2 * SUPER : 4 * SUPER].rearrange("p (k g) -> p k g", k=2), Exp
            )
            den = small.tile([P, SUPER], f32, tag="den")
            nc.vector.tensor_tensor(den, e0, e12[:, 0, :], add)
            nc.vector.tensor_tensor(den, den, e12[:, 1, :], add)
            rden = small.tile([P, SUPER], f32, tag="rden")
            nc.vector.reciprocal(rden, den)
            b12 = small.tile([P, 2, SUPER], f32, tag="b12")
            nc.vector.tensor_tensor(b12[:, 0, :], e12[:, 0, :], rden, mult)
            nc.vector.tensor_tensor(b12[:, 1, :], e12[:, 1, :], rden, mult)

            # ---- phase C: neighbor gather (paired f32r matmuls) + combine ----
            o_sb = opool.tile([P, G, D], f32, tag="o")
            for pr in range((G + 1) // 2):
                m = min(2, G - pr * 2)
                gg = pr * 2
                zsh1 = ps_zsh1.tile([P, 2, D], f32, tag="zsh1")
                zsh2 = ps_zsh2.tile([P, 2, D], f32, tag="zsh2")
                nc.tensor.matmul(
                    zsh1[:, 0:m, :], g1_sb, z_sb[:, gg : gg + m, :],
                    start=True, stop=True,
                )
                nc.tensor.matmul(
                    zsh2[:, 0:m, :], g2_sb, z_sb[:, gg : gg + m, :],
                    start=True, stop=True,
                )
                for pm in range(m):
                    g = gg + pm
                    # t2 = beta2 * z[j2]  (ScalarE evacuates one psum term)
                    t2 = tzpool.tile([P, D], f32, tag="t2")
                    nc.scalar.activation(
                        t2, zsh2[:, pm, :], Copy, scale=b12[:, 1, g : g + 1]
                    )
                    # u = beta1 * z[j1] + z   (exact fp32 residual)
                    u = tzpool.tile([P, D], f32, tag="u")
                    nc.vector.scalar_tensor_tensor(
                        u,
                        zsh1[:, pm, :],
                        b12[:, 0, g : g + 1],
                        z_sb[:, g, :],
                        op0=mult,
                        op1=add,
                    )
                    nc.vector.tensor_tensor(o_sb[:, g, :], u, t2, add)
            nc.sync.dma_start(
                out=o_d[row0 : row0 + G * P, :].rearrange("(g p) d -> p g d", p=P),
                in_=o_sb,
            )

    nc.finalize()
    return nc


_NC_CACHE = None


def _get_nc():
    global _NC_CACHE
    if _NC_CACHE is None:
        _NC_CACHE = _build_nc()
    return _NC_CACHE


def _prepare_in_maps(z, W1, W2):
    z = np.asarray(z, dtype=np.float32)
    zp = np.zeros((N_PAD, R, D), np.float32)
    zp[: z.shape[0]] = z

    g1, g2, w1t, w2t = _build_consts(
        np.asarray(W1, np.float32), np.asarray(W2, np.float32)
    )
    in_maps = []
    for c in range(N_CORES):
        sl = slice(c * NODES_PER_CORE, (c + 1) * NODES_PER_CORE)
        in_maps.append(
            {
                "z": np.ascontiguousarray(zp[sl].reshape(ROWS_PER_CORE, D)).astype(
                    np.float16
                ),
                "zt": np.ascontiguousarray(
                    zp[sl].reshape(ROWS_PER_CORE, 2, P).transpose(1, 2, 0)
                ).astype(np.float16),
                "g1": g1,
                "g2": g2,
                "w1t": w1t,
                "w2t": w2t,
            }
        )
    return in_maps


def _gather_out(res, n):
    out = np.empty((N_PAD, R, D), np.float32)
    for c in range(N_CORES):
        out[c * NODES_PER_CORE : (c + 1) * NODES_PER_CORE] = res.results[c][
            "o"
        ].reshape(NODES_PER_CORE, R, D)
    return out[:n]


def kernel(z, W1, W2):
    from concourse.bass_utils import run_bass_kernel_spmd

    nc = _get_nc()
    in_maps = _prepare_in_maps(z, W1, W2)
    res = run_bass_kernel_spmd(nc, in_maps, core_ids=list(range(N_CORES)))
    return _gather_out(res, np.asarray(z).shape[0])


# revision 14
# speedup vs baseline: 1.0208x; 1.0208x over previous
"""Trainium2 Bass kernel for nn_Attentioncross (gnn_message_passing).

Reference computation, per node n (N=50000) and row r (R=8), D=256:
    idx[r] = [r, r+1, r-1] (with idx[0]=[0,1,2], idx[7]=[7,6,5])
    s[n,j]   = W2 @ leaky_relu(W1 @ z[n,j,:], 0.01)        (scalar per row)
    beta     = softmax([s[self], s[j1], s[j2]])            (over the 3)
    o[n,r,:] = z[n,r,:] + beta1*z[n,j1,:] + beta2*z[n,j2,:]

Strategy: data-parallel over N across 8 cores. Per core, rows are tiled
[128, 256] (16 nodes/tile; the neighbor structure is block-diagonal per
node, so intra-tile). The score path consumes a bf16 copy of z loaded
pre-transposed via the DMA xbar (d on partitions), so no PE transposes
are needed: h^T = W1 @ z^T as bf16 matmuls, leaky on ScalarE, then
s = h^T.T @ W2^T gives per-row scores on partitions. Neighbor gathers of
scores and of z-rows are f32r matmuls against static block-diagonal 0/1
shift matrices; softmax runs on [128, 8] vectors; the combine applies
per-row betas via scalar_tensor_tensor / scaled-copy with the fp32
residual added exactly.
"""
import sys

for p in ("/opt/trn_rl_repo",):
    if p not in sys.path:
        sys.path.insert(0, p)

import numpy as np
from contextlib import ExitStack

N_FULL, R, D = 50000, 8, 256
N_CORES = 8
NODES_PER_TILE = 16          # 128 rows / 8
P = 128
TILES_PER_CORE = 391         # 6256 nodes/core * 8 rows / 128
NODES_PER_CORE = TILES_PER_CORE * NODES_PER_TILE   # 6256
N_PAD = NODES_PER_CORE * N_CORES                   # 50048
ROWS_PER_CORE = NODES_PER_CORE * R                 # 50048
SUPER = 8                    # tiles per supertile

# static 3-neighbor pattern (matches reference._neighbor_idx for R=8)
J1 = [1, 2, 3, 4, 5, 6, 7, 6]
J2 = [2, 0, 1, 2, 3, 4, 5, 5]


def _build_consts(W1, W2):
    g1 = np.zeros((P, P), np.float16)
    g2 = np.zeros((P, P), np.float16)
    for b in range(NODES_PER_TILE):
        for r in range(R):
            g1[b * R + J1[r], b * R + r] = 1.0
            g2[b * R + J2[r], b * R + r] = 1.0
    w1t = np.ascontiguousarray(
        W1.T.reshape(2, P, 16).transpose(1, 0, 2)
    ).astype(np.float16)  # [128, 2, 16]
    w2t = np.ascontiguousarray(
        np.repeat(W2.reshape(16, 1), 2, axis=1)
    ).astype(np.float16)  # [16, 2]
    return g1, g2, w1t, w2t


def _build_nc():
    import concourse.bacc as bacc
    import concourse.tile as tile
    from concourse import mybir

    f32 = mybir.dt.float32
    f32r = mybir.dt.float32r
    f16 = mybir.dt.float16

    nc = bacc.Bacc("TRN2", target_bir_lowering=False)
    z_d = nc.declare_dram_parameter("z", [ROWS_PER_CORE, D], f16, isOutput=False)
    zt_d = nc.declare_dram_parameter("zt", [2, P, ROWS_PER_CORE], f16, isOutput=False)
    g1_d = nc.declare_dram_parameter("g1", [P, P], f16, isOutput=False)
    g2_d = nc.declare_dram_parameter("g2", [P, P], f16, isOutput=False)
    w1t_d = nc.declare_dram_parameter("w1t", [P, 2, 16], f16, isOutput=False)
    w2t_d = nc.declare_dram_parameter("w2t", [16, 2], f16, isOutput=False)
    o_d = nc.declare_dram_parameter("o", [ROWS_PER_CORE, D], f32, isOutput=True)

    Prelu = mybir.ActivationFunctionType.Prelu
    Exp = mybir.ActivationFunctionType.Exp
    Copy = mybir.ActivationFunctionType.Copy
    add = mybir.AluOpType.add
    mult = mybir.AluOpType.mult

    with tile.TileContext(nc) as tc, ExitStack() as ctx:
        consts = ctx.enter_context(tc.tile_pool(name="consts", bufs=1))
        zpool = ctx.enter_context(tc.tile_pool(name="zp", bufs=4))
        opool = ctx.enter_context(tc.tile_pool(name="op", bufs=3))
        zt_pool = ctx.enter_context(tc.tile_pool(name="ztp", bufs=3))
        small = ctx.enter_context(tc.tile_pool(name="small", bufs=3))
        tzpool = ctx.enter_context(tc.tile_pool(name="tzp", bufs=6))

        ps_ht = ctx.enter_context(tc.tile_pool(name="ps_ht", bufs=1, space="PSUM"))
        ps_sc = ctx.enter_context(tc.tile_pool(name="ps_sc", bufs=1, space="PSUM"))
        ps_zsh1 = ctx.enter_context(tc.tile_pool(name="ps_zsh1", bufs=3, space="PSUM"))
        ps_zsh2 = ctx.enter_context(tc.tile_pool(name="ps_zsh2", bufs=3, space="PSUM"))

        g1_sb = consts.tile([P, P], f16)
        g2_sb = consts.tile([P, P], f16)
        w1t_sb = consts.tile([P, 2, 16], f16)
        w2t_sb = consts.tile([16, 2], f16)
        nc.sync.dma_start(out=g1_sb, in_=g1_d[:])
        nc.sync.dma_start(out=g2_sb, in_=g2_d[:])
        nc.sync.dma_start(out=w1t_sb, in_=w1t_d[:])
        nc.sync.dma_start(out=w2t_sb, in_=w2t_d[:])

        n_super = (TILES_PER_CORE + SUPER - 1) // SUPER
        for st in range(n_super):
            g0 = st * SUPER
            G = min(SUPER, TILES_PER_CORE - g0)
            row0 = g0 * P

            z_sb = zpool.tile([P, G, D], f16, tag="z")
            nc.sync.dma_start(
                out=z_sb,
                in_=z_d[row0 : row0 + G * P, :].rearrange(
                    "(g p) d -> p g d", p=P
                ),
            )
            # fp16 z^T for the score path (host-pretransposed, contiguous)
            zt_sb = zt_pool.tile([P, 2, SUPER * P], f16, tag="zt")
            nc.sync.dma_start(
                out=zt_sb[:, :, 0 : G * P],
                in_=zt_d[:, :, row0 : row0 + G * P].rearrange("c p r -> p c r"),
            )

            # ---- phase A: scores ----
            nquad = (G + 3) // 4
            ht_sbs = []
            for q in range(nquad):
                m = min(4, G - q * 4)  # tiles in this quad
                ht_ps = ps_ht.tile([16, 4, P], f32, tag="ht")
                for c in range(2):
                    nc.tensor.matmul(
                        ht_ps[:, 0:m, :],
                        w1t_sb[:, c, :],
                        zt_sb[:, c, q * 4 * P : (q * 4 + m) * P],
                        start=(c == 0),
                        stop=(c == 1),
                    )
                ht_sb = small.tile([16, 4, P], f16, tag=f"ht{q % 2}")
                nc.scalar.activation(
                    ht_sb[:, 0:m, :], ht_ps[:, 0:m, :], Prelu, alpha=0.01
                )
                ht_sbs.append((ht_sb, m))

            # scores psum bank: cols 0:16 = s ([8 tiles, 2]), 16:32 = s1/s2
            sc_ps = ps_sc.tile([P, 2 * SUPER + 2 * SUPER], f32, tag="sc")
            for q, (ht_sb, m) in enumerate(ht_sbs):
                for t in range(m):
                    g = q * 4 + t
                    nc.tensor.matmul(
                        sc_ps[:, 2 * g : 2 * g + 2],
                        ht_sb[:, t, :],
                        w2t_sb,
                        start=True,
                        stop=True,
                    )
            s_sb = small.tile([P, SUPER], f16, tag="ssb")
            nc.vector.tensor_copy(
                s_sb,
                sc_ps[:, 0 : 2 * SUPER].rearrange("p (g two) -> p g two", two=2)[
                    :, :, 0
                ],
            )
            nc.tensor.matmul(sc_ps[:, 2 * SUPER : 3 * SUPER], g1_sb, s_sb, start=True, stop=True)
            nc.tensor.matmul(sc_ps[:, 3 * SUPER : 4 * SUPER], g2_sb, s_sb, start=True, stop=True)

            # softmax over {self, n1, n2}; no max-subtraction (|s| < ~8)
            e0 = small.tile([P, SUPER], f32, tag="e0")
            e12 = small.tile([P, 2, SUPER], f32, tag="e12")
            nc.scalar.activation(e0, s_sb, Exp)
            nc.scalar.activation(
                e12, sc_ps[:, 2 * SUPER : 4 * SUPER].rearrange("p (k g) -> p k g", k=2), Exp
            )
            den = small.tile([P, SUPER], f32, tag="den")
            nc.vector.tensor_tensor(den, e0, e12[:, 0, :], add)
            nc.vector.tensor_tensor(den, den, e12[:, 1, :], add)
            rden = small.tile([P, SUPER], f32, tag="rden")
            nc.vector.reciprocal(rden, den)
            b12 = small.tile([P, 2, SUPER], f32, tag="b12")
            nc.vector.tensor_tensor(b12[:, 0, :], e12[:, 0, :], rden, mult)
            nc.vector.tensor_tensor(b12[:, 1, :], e12[:, 1, :], rden, mult)

            # ---- phase C: neighbor gather (paired f32r matmuls) + combine ----
            o_sb = opool.tile([P, G, D], f32, tag="o")
            for pr in range((G + 1) // 2):
                m = min(2, G - pr * 2)
                gg = pr * 2
                zsh1 = ps_zsh1.tile([P, 2, D], f32, tag="zsh1")
                zsh2 = ps_zsh2.tile([P, 2, D], f32, tag="zsh2")
                nc.tensor.matmul(
                    zsh1[:, 0:m, :], g1_sb, z_sb[:, gg : gg + m, :],
                    start=True, stop=True,
                )
                nc.tensor.matmul(
                    zsh2[:, 0:m, :], g2_sb, z_sb[:, gg : gg + m, :],
                    start=True, stop=True,
                )
                for pm in range(m):
                    g = gg + pm
                    # t2 = beta2 * z[j2]  (ScalarE evacuates one psum term)
                    t2 = tzpool.tile([P, D], f32, tag="t2")
                    nc.scalar.activation(
                        t2, zsh2[:, pm, :], Copy, scale=b12[:, 1, g : g + 1]
                    )
                    # u = beta1 * z[j1] + z   (exact fp32 residual)
                    u = tzpool.tile([P, D], f32, tag="u")
                    nc.vector.scalar_tensor_tensor(
                        u,
                        zsh1[:, pm, :],
                        b12[:, 0, g : g + 1],
                        z_sb[:, g, :],
                        op0=mult,
                        op1=add,
                    )
                    nc.vector.tensor_tensor(o_sb[:, g, :], u, t2, add)
            nc.sync.dma_start(
                out=o_d[row0 : row0 + G * P, :].rearrange("(g p) d -> p g d", p=P),
                in_=o_sb,
            )

    nc.finalize()
    return nc


_NC_CACHE = None


def _get_nc():
    global _NC_CACHE
    if _NC_CACHE is None:
        _NC_CACHE = _build_nc()
    return _NC_CACHE


def _prepare_in_maps(z, W1, W2):
    z = np.asarray(z, dtype=np.float32)
    zp = np.zeros((N_PAD, R, D), np.float32)
    zp[: z.shape[0]] = z

    g1, g2, w1t, w2t = _build_consts(
        np.asarray(W1, np.float32), np.asarray(W2, np.float32)
    )
    in_maps = []
    for c in range(N_CORES):
        sl = slice(c * NODES_PER_CORE, (c + 1) * NODES_PER_CORE)
        in_maps.append(
            {
                "z": np.ascontiguousarray(zp[sl].reshape(ROWS_PER_CORE, D)).astype(
                    np.float16
                ),
                "zt": np.ascontiguousarray(
                    zp[sl].reshape(ROWS_PER_CORE, 2, P).transpose(1, 2, 0)
                ).astype(np.float16),
                "g1": g1,
                "g2": g2,
                "w1t": w1t,
                "w2t": w2t,
            }
        )
    return in_maps


def _gather_out(res, n):
    out = np.empty((N_PAD, R, D), np.float32)
    for c in range(N_CORES):
        out[c * NODES_PER_CORE : (c + 1) * NODES_PER_CORE] = res.results[c][
            "o"
        ].reshape(NODES_PER_CORE, R, D)
    return out[:n]


def kernel(z, W1, W2):
    from concourse.bass_utils import run_bass_kernel_spmd

    nc = _get_nc()
    in_maps = _prepare_in_maps(z, W1, W2)
    res = run_bass_kernel_spmd(nc, in_maps, core_ids=list(range(N_CORES)))
    return _gather_out(res, np.asarray(z).shape[0])
